# revision 1
# baseline (speedup 1.0000x reference)
"""Trainium2 Bass kernel for nn_Attention (B=4, N=2048, C=768, H=12).

Sharding: 8 cores = 4 batches x 2 head-groups (6 heads each), Megatron-style
tensor parallel on the heads. Each core computes qkv for its head slice,
attention for 6 heads, and per-head-pair output-projection partials
out3 [3, 2048, 768]. The host sums the 3 pair partials of the 2 cores
covering each batch and adds the bias.

Per-core attention scheme (no transposes anywhere):
  - q,k stored [d, n] (feature-major) straight out of the QKV matmul; heads
    packed in pairs per 128-partition group (head 2p -> partitions 0-63,
    head 2p+1 -> 64-127).
  - S^T tiles [128 j, i] = k_chunk.T @ q  (K=64 matmul). exp() on scalar
    engine reads PSUM, writes SBUF. No max subtraction (logits are O(10);
    softmax is shift-invariant so this only perturbs rounding).
  - v stored [n, d] with an extra ones column; PV matmul lhsT=v[j,0:65],
    rhs=exp(S^T) accumulates [65, 512] where row 64 = sum_j exp = Z.
  - normalize: 1/Z broadcast across partitions via a DRAM-bounce DMA, one
    DVE multiply; odd heads' results are DMA'd up to partitions 64-127
    (engines cannot shift partitions; DMA can).

Matmuls default to float32r (TF32-class, 4x faster than fp32 on the PE;
measured 3.8e-4 scale-relative absmax error vs the fp32 reference).
Set KERNEL_MM_DT=float32 for full fp32 precision (3.4e-6) at ~3x the time.
"""

import os
import sys
from contextlib import ExitStack

if "/opt/trn_rl_repo" not in sys.path:
    sys.path.insert(0, "/opt/trn_rl_repo")

import numpy as np

import concourse.bass as bass
import concourse.mybir as mybir
import concourse.tile as tile
from concourse import bass_utils

F32 = mybir.dt.float32

B, N, C = 4, 2048, 768
NH, D = 12, 64
SCALE = D ** -0.5
HPC = NH // 2          # heads per core
F = HPC * D            # 384 per-core features per projection
QKVF = 3 * F           # 1152
P = 128
CO = C // P            # 6 contraction chunks
FO = F // P            # 3 feature chunks (head pairs)
NO = N // P            # 16 token chunks of 128
NCORES = 8

_MM_DT_NAME = os.environ.get("KERNEL_MM_DT", "float32r")
MM_DT = getattr(mybir.dt, _MM_DT_NAME)


def _d(ap):
    """Cast an fp32 AP to the matmul compute dtype (bitcast, same bytes)."""
    return ap.bitcast(MM_DT) if MM_DT != F32 else ap


def _r(ap):
    """Cast a producer OUT AP feeding a matmul to the compute dtype, so the
    producing engine rounds to fp32r (walrus verifies this chain)."""
    return ap.bitcast(MM_DT) if MM_DT == mybir.dt.float32r else ap


def _split_multiwaits(nc):
    """This container's walrus accepts at most ONE sync-wait per instruction.

    Split any instruction carrying N>1 waits into (N-1) single-wait NOPs on
    the same engine queue placed immediately before it (engine queues are
    FIFO, so the semantics are identical)."""
    ctr = 0
    for f in nc.m.functions:
        for blk in f.blocks:
            insts = blk.instructions
            out = []
            changed = False
            for ins in insts:
                si = ins.sync_info
                if si is not None and len(si.on_wait) > 1:
                    changed = True
                    waits = list(si.on_wait)
                    for ww in waits[:-1]:
                        nop = mybir.InstNoOp(name=f"zzsplitw_{ctr}", ins=[], outs=[])
                        ctr += 1
                        nop.engine = ins.engine
                        nop.sync_info = mybir.SyncInfo(on_wait=[ww], on_update=[])
                        out.append(nop)
                    ins.sync_info = mybir.SyncInfo(
                        on_wait=[waits[-1]], on_update=list(si.on_update)
                    )
                out.append(ins)
            if changed:
                blk.instructions = out
    return nc


def _emit(nc, tc, ctx):
    # x pre-chunked host-side to [co][n4][128, 512] so every slice DMA is
    # one fully-contiguous 256KB read
    xTc = nc.dram_tensor("xTc", [CO, 4, P, 512], F32, kind="ExternalInput").ap()
    # five contiguous weight sections (fully linear DMA reads; a single
    # [C, 1152] tensor would make every section load a 512B-strided gather
    # during the bandwidth-bound lead-in)
    wq_secs = {
        lo: nc.dram_tensor(f"wq{lo}", [C, hi - lo], F32, kind="ExternalInput").ap()
        for lo, hi in ((0, P), (F, F + P), (2 * F, 3 * F), (P, F), (F + P, 2 * F))
    }
    wprojT = nc.dram_tensor("wprojT", [F, C], F32, kind="ExternalInput").ap()
    out3 = nc.dram_tensor("out3", [FO, N, C], F32, kind="ExternalOutput").ap()

    persist = ctx.enter_context(tc.tile_pool(name="persist", bufs=1))

    # q/k in [feature, token] layout, split per (pair, 512-token chunk) so
    # consumers wait only on the producer they actually need (Tile tracks
    # dependencies at whole-tile granularity).
    q_sb = [[persist.tile([P, 512], F32, tag=f"q{fo}_{n4}", name=f"q{fo}_{n4}")
             for n4 in range(4)] for fo in range(FO)]
    k_sb = [[persist.tile([P, 512], F32, tag=f"k{fo}_{n4}", name=f"k{fo}_{n4}")
             for n4 in range(4)] for fo in range(FO)]
    # v in [token, feature] layout per 128-token chunk, +1 ones column.
    v_sb = [persist.tile([P, HPC, D + 1], F32, tag=f"v{no}", name=f"v{no}") for no in range(NO)]
    # attention output per pair, [feature, token] layout; 2 rotating slots
    # (pair 2 reuses pair 0's slot once proj-0 has drained it)
    otp = ctx.enter_context(tc.tile_pool(name="otp", bufs=2))
    ot_sb = [otp.tile([P, N], F32, tag="ot", name=f"ot{pr}") for pr in range(FO)]
    wp_sb = persist.tile([P, FO, C], F32, tag="wp")

    ones_sb = persist.tile([P, HPC], F32, tag="ones")
    nc.vector.memset(ones_sb, 1.0)
    for no in range(NO):
        # DVE copy (not memset) so the output can be declared fp32r
        nc.vector.tensor_copy(out=_r(v_sb[no][:, :, D : D + 1]), in_=ones_sb)
    # dummy exp: pulls the ~2.7us ACT table load into the DMA lead-in window
    expwarm = persist.tile([P, HPC], F32, tag="expwarm")
    nc.scalar.activation(
        out=expwarm,
        in_=ones_sb,
        func=mybir.ActivationFunctionType.Exp,
        scale=1.0,
    )

    with (
        tc.tile_pool(name="wqp", bufs=1) as wqp,
        tc.tile_pool(name="xs", bufs=4) as xs_pool,
        tc.tile_pool(name="ptp", bufs=3) as pt_pool,
        tc.tile_pool(name="rp", bufs=2) as r_pool,
        tc.tile_pool(name="outp", bufs=3) as outp,
        tc.tile_pool(name="rd", bufs=3, space="DRAM") as rd_pool,
        tc.tile_pool(name="ps1", bufs=2, space="PSUM") as ps1,
        tc.tile_pool(name="ps_st", bufs=2, space="PSUM") as ps_st,
        tc.tile_pool(name="ps_o", bufs=2, space="PSUM") as ps_o,
    ):
        # weight tiles per (column-section, contraction chunk) so each qkv
        # matmul depends on exactly one DMA
        wq_tiles = {}

        def load_wq(slices):
            for lo, hi in slices:
                for co in range(CO):
                    t = wqp.tile([P, hi - lo], F32, tag=f"wq_{lo}_{co}",
                                 name=f"wq_{lo}_{co}")
                    wq_tiles[(lo, co)] = t
                    nc.sync.dma_start(
                        out=_r(t),
                        in_=_r(wq_secs[lo][co * P : (co + 1) * P, :]),
                    )

        def wq_slice(foff, co, width=P):
            """AP for weight columns [foff, foff+width) of chunk co."""
            for lo, hi in ((0, P), (F, F + P), (2 * F, 3 * F), (P, F), (F + P, 2 * F)):
                if lo <= foff and foff + width <= hi:
                    return wq_tiles[(lo, co)][:, foff - lo : foff - lo + width]
            raise KeyError(foff)

        def emit_qkv_pass(fo, pre_xt4=None):
            """q/k chunk fo over all tokens.

            Streams xT per 512-token slice (xT is re-read from DRAM once per
            pass; DMA is far from the bottleneck and this keeps SBUF free)."""
            xt4s = []
            for n4 in range(4):
                if n4 == 0 and pre_xt4 is not None:
                    # tile + DMAs already emitted (interleaved with weights);
                    # still run this n4's compute below
                    xt4 = pre_xt4
                    xt4s.append(xt4)
                else:
                    xt4 = xs_pool.tile([P, CO, 512], F32, tag="xt4",
                                       name=f"xt4_{fo}_{n4}")
                    xt4s.append(xt4)
                    # one DMA per contraction chunk so the first matmul can
                    # start after ~256KB instead of the full 1.5MB slice
                    for co in range(CO):
                        nc.sync.dma_start(
                            out=_r(xt4[:, co, :]),
                            in_=_r(xTc[co, n4, :, :]),
                        )
                for dst, foff in ((q_sb[fo][n4], fo * P), (k_sb[fo][n4], F + fo * P)):
                    pq = ps1.tile([P, 512], F32, tag="pqk")
                    for co in range(CO):
                        nc.tensor.matmul(
                            pq,
                            _d(wq_slice(foff, co)),
                            _d(xt4[:, co, :]),
                            start=(co == 0),
                            stop=(co == CO - 1),
                        )
                    nc.vector.tensor_copy(out=_r(dst), in_=pq)
            return xt4s

        def emit_v_chunk(no, xtv):
            """v for one 128-token chunk, reading an [P, CO, 512] x-slice."""
            pv = ps1.tile([P, F], F32, tag="pqk", name=f"pv_{no}")
            for co in range(CO):
                nc.tensor.matmul(
                    pv,
                    _d(xtv[:, co, (no % 4) * P : (no % 4 + 1) * P]),
                    _d(wq_slice(2 * F, co, F)),
                    start=(co == 0),
                    stop=(co == CO - 1),
                )
            nc.vector.tensor_copy(
                out=_r(v_sb[no][:, :, 0:D]),
                in_=pv.rearrange("p (h d) -> p h d", h=HPC),
            )

        def emit_normalize(po, pr, plo, i512):
            # evacuate PSUM -> SBUF at once so the po slot frees for the next
            # i512 block (the normalize chain below has DMA latency in it)
            ov = r_pool.tile([65, 512], F32, tag="ov", name=f"ov_{pr}_{plo}_{i512}")
            # 1/Z lives on partition 64 (engines cannot move data across
            # partitions, so compute in place on lane 64); reading po directly
            # lets the broadcast DMA start before the row evacuation finishes
            nc.vector.reciprocal(out=ov[64:65, :], in_=po[64:65, :])
            nc.vector.tensor_copy(out=ov[0:64, :], in_=po[0:64, :])
            # partition-broadcast 1/Z: SBUF zero-step partition APs are
            # illegal, so bounce through DRAM (DRAM APs broadcast fine)
            rdram = rd_pool.tile([1, 512], F32, tag="rd", name=f"rd_{pr}_{plo}_{i512}")
            nc.sync.dma_start(out=rdram, in_=ov[64:65, :])
            rb = r_pool.tile([64, 512], F32, tag="rb", name=f"rb_{pr}_{plo}_{i512}")
            nc.sync.dma_start(out=rb, in_=rdram.to_broadcast([64, 512]))
            if plo == 0:
                nc.vector.tensor_mul(
                    out=_r(ot_sb[pr][0:64, i512 : i512 + 512]),
                    in0=ov[0:64, :],
                    in1=rb,
                )
            else:
                # odd head: normalize at partitions 0-63, then DMA up to
                # partitions 64-127 of ot
                nt = r_pool.tile([64, 512], F32, tag="nt", name=f"nt_{pr}_{i512}")
                nc.vector.tensor_mul(out=_r(nt), in0=ov[0:64, :], in1=rb)
                nc.sync.dma_start(
                    out=_r(ot_sb[pr][64:128, i512 : i512 + 512]), in_=_r(nt)
                )

        # interleave: qkv pass for a head pair, then that pair's attention.
        # Both heads of a pair share one [128, 1024] S^T tile (head A cols
        # 0-511, head B cols 512-1023): their K=64 matmuls sit at PE row
        # groups 0-1 / 2-3 and run concurrently, and one exp() covers both.
        def emit_attention(pr, interleave_proj=False, xt4s=None):
            hA, hB = 2 * pr, 2 * pr + 1
            for i4 in range(4):
                i0 = i4 * 512
                po_A = ps_o.tile([65, 512], F32, tag="po", name=f"poA_{pr}_{i4}")
                po_B = ps_o.tile([65, 512], F32, tag="po", name=f"poB_{pr}_{i4}")
                for j in range(NO):
                    kt = k_sb[pr][j // 4]
                    jo = (j % 4) * P
                    qt = q_sb[pr][i4]
                    stm = ps_st.tile([P, 1024], F32, tag="st", name=f"st_{j}")
                    nc.tensor.matmul(
                        stm[:, 0:512],
                        _d(kt[0:64, jo : jo + P]),
                        _d(qt[0:64, :]),
                        start=True,
                        stop=True,
                    )
                    nc.tensor.matmul(
                        stm[:, 512:1024],
                        _d(kt[64:128, jo : jo + P]),
                        _d(qt[64:128, :]),
                        start=True,
                        stop=True,
                    )
                    ptile = pt_pool.tile([P, 1024], F32, tag="pt", name=f"pt_{j}")
                    nc.scalar.activation(
                        out=_r(ptile),
                        in_=stm,
                        func=mybir.ActivationFunctionType.Exp,
                        scale=SCALE,
                    )
                    if xt4s is not None and i4 == 0:
                        # produce v[j] just before its first consumer, reusing
                        # the x slices already in SBUF from the q/k pass; these
                        # matmuls fill PE gaps while the scalar engine exps
                        emit_v_chunk(j, xt4s[j // 4])
                    nc.tensor.matmul(
                        po_A,
                        _d(v_sb[j][:, hA, :]),
                        _d(ptile[:, 0:512]),
                        start=(j == 0),
                        stop=(j == NO - 1),
                    )
                    nc.tensor.matmul(
                        po_B,
                        _d(v_sb[j][:, hB, :]),
                        _d(ptile[:, 512:1024]),
                        start=(j == 0),
                        stop=(j == NO - 1),
                    )
                emit_normalize(po_A, pr, 0, i0)
                emit_normalize(po_B, pr, 64, i0)
                if interleave_proj:
                    emit_proj(pr, no_range=range(4 * i4, 4 * i4 + 4))

        def emit_proj(pr, no_range=None):
            # per-pair projection partial: out3[pr] = ot_pair.T @ wp[pr]
            # (the host sums the three pair partials; this removes the
            # cross-pair barrier and overlaps proj with the next pair)
            for no in (no_range if no_range is not None else range(NO)):
                o_sb = outp.tile([P, C], F32, tag="o", name=f"o_{pr}_{no}")
                for ob, width in ((0, 512), (1, 256)):
                    pp = ps1.tile([P, 512], F32, tag="pqk", name=f"pp_{pr}_{no}_{ob}")
                    nc.tensor.matmul(
                        pp[:, 0:width],
                        _d(ot_sb[pr][:, no * P : (no + 1) * P]),
                        _d(wp_sb[:, pr, ob * 512 : ob * 512 + width]),
                        start=True,
                        stop=True,
                    )
                    nc.vector.tensor_copy(
                        out=o_sb[:, ob * 512 : ob * 512 + width], in_=pp[:, 0:width]
                    )
                nc.sync.dma_start(
                    out=out3[pr, no * P : (no + 1) * P, :], in_=o_sb
                )

        # emission order = scheduling priority. Minimal weights first so
        # compute starts ~10us in; qkv pass pr runs in PE slack during
        # attention pr-1; proj pr-1 runs during attention pr; the last
        # pair's proj interleaves into its own attention blocks.
        # interleave q0/k0 weight DMAs with the first x slice per chunk so
        # the first matmul's operands co-arrive in the DMA queue
        xt4_00 = xs_pool.tile([P, CO, 512], F32, tag="xt4", name="xt4_00")
        for co in range(CO):
            for lo, hi in ((0, P), (F, F + P)):
                t = wqp.tile([P, hi - lo], F32, tag=f"wq_{lo}_{co}",
                             name=f"wq_{lo}_{co}")
                wq_tiles[(lo, co)] = t
                nc.sync.dma_start(
                    out=_r(t), in_=_r(wq_secs[lo][co * P : (co + 1) * P, :])
                )
            nc.sync.dma_start(
                out=_r(xt4_00[:, co, :]), in_=_r(xTc[co, 0, :, :])
            )
        xt4s0 = emit_qkv_pass(0, pre_xt4=xt4_00)
        load_wq([(2 * F, 3 * F)])                      # v (needed ~12us in)
        emit_attention(0, xt4s=xt4s0)
        load_wq([(P, F), (F + P, 2 * F)])              # q1/q2, k1/k2
        for fo in range(FO):
            nc.sync.dma_start(
                out=_r(wp_sb[:, fo, :]),
                in_=_r(wprojT[fo * P : (fo + 1) * P, :]),
            )
        for pr in range(1, FO):
            emit_qkv_pass(pr)
            emit_proj(pr - 1)
            emit_attention(pr, interleave_proj=(pr == FO - 1))


_NC_CACHE = {}


def build_bass():
    key = _MM_DT_NAME
    if key in _NC_CACHE:
        return _NC_CACHE[key]
    nc = bass.Bass("TRN2")
    with tile.TileContext(nc) as tc:
        with ExitStack() as ctx:
            _emit(nc, tc, ctx)
    _split_multiwaits(nc)
    _NC_CACHE[key] = nc
    return nc


def make_in_maps(x, w_qkv, w_proj):
    x = np.asarray(x, dtype=np.float32)
    w_qkv = np.asarray(w_qkv, dtype=np.float32)
    w_proj = np.asarray(w_proj, dtype=np.float32)
    wq, wk, wv = w_qkv[0:C], w_qkv[C : 2 * C], w_qkv[2 * C : 3 * C]
    in_maps = []
    for c in range(NCORES):
        b, g = divmod(c, 2)
        sl = slice(g * F, (g + 1) * F)
        wslice = np.concatenate([wq[sl], wk[sl], wv[sl]], axis=0)  # [1152, 768]
        wT = np.ascontiguousarray(wslice.T)  # [768, 1152]
        xT = x[b].T  # [768, 2048]
        xTc = np.ascontiguousarray(
            xT.reshape(CO, P, 4, 512).transpose(0, 2, 1, 3)
        )  # [co, n4, 128, 512]
        m = {
            "xTc": xTc,
            "wprojT": np.ascontiguousarray(w_proj[:, sl].T),
        }
        for lo, hi in ((0, 128), (384, 512), (768, 1152), (128, 384), (512, 768)):
            m[f"wq{lo}"] = np.ascontiguousarray(wT[:, lo:hi])
        in_maps.append(m)
    return in_maps


def gather_output(parts, b_proj):
    """parts: 8 arrays [FO, N, C] (pair partials per core)."""
    outv = np.empty((B, N, C), np.float32)
    for b in range(B):
        outv[b] = parts[2 * b].sum(axis=0) + parts[2 * b + 1].sum(axis=0)
    outv += np.asarray(b_proj, dtype=np.float32)[None, None, :]
    return outv


def kernel(x, w_qkv, w_proj, b_proj, _run_kwargs=None):
    nc = build_bass()
    in_maps = make_in_maps(x, w_qkv, w_proj)
    res = bass_utils.run_bass_kernel_spmd(
        nc, in_maps, core_ids=list(range(NCORES)), **(_run_kwargs or {})
    )
    parts = [r["out3"] for r in res.results]
    outv = gather_output(parts, b_proj)
    if _run_kwargs is not None:
        kernel.last_results = res
    return outv



# revision 5
# speedup vs baseline: 1.1323x; 1.1323x over previous
"""Trainium2 Bass kernel for nn_Attention (B=4, N=2048, C=768, H=12).

Sharding: 8 cores = 4 batches x 2 head-groups (6 heads = 3 pairs each),
Megatron-style tensor parallel on heads. Each core computes qkv for its head
slice, attention, and the head-group's projection partial out [2048, 768];
the host sums the 2 group partials per batch and adds the bias.

Design (cost-model driven; matmul cost = out_free_size x 1 cycle/row for f16):
  - All matmul operands fp16 (host-cast inputs); PSUM accumulation fp32.
    Measured end-to-end rel err ~1e-3 vs fp32 reference (tolerance 2e-2).
  - x resident in SBUF as 6 [128, 2048] c-chunks, read from DRAM once.
  - S^T tiles [128 j, 1024] = two heads x 512 i, exp'd on the Act engine
    straight out of PSUM into fp16 ptiles.
  - PV in [i, d] layout: po[i, 65] += ptile[j, i-block].T @ v[j, d+ones]
    (free size 65 instead of 512 -> half the PE cost of a [d, i] PV).
    Column 64 of v is ones so row sums Z accumulate alongside.
  - PV for window (pair, i4) is deferred one window: it fills PE slack while
    the next window's exp stream keeps the Act engine (the near-critical
    engine) saturated.
  - Normalize: 1/Z per-partition (token) via DVE reciprocal + tensor_tensor
    broadcast multiply -> [i, d] fp16; PE-transpose (identity) -> ot [d, i];
    projection accumulates all 3 pairs into one PSUM tile per token block.
  - qkv / v / proj / transpose work is emitted via a deadline-driven filler
    queue into the attention windows' PE slack.
"""

import os
import sys
from contextlib import ExitStack

if "/opt/trn_rl_repo" not in sys.path:
    sys.path.insert(0, "/opt/trn_rl_repo")

import numpy as np

import concourse.bass as bass
import concourse.mybir as mybir
import concourse.tile as tile
from concourse import bass_utils

F32 = mybir.dt.float32
F16 = mybir.dt.float16
I16 = mybir.dt.int16

B, N, C = 4, 2048, 768
NH, D = 12, 64
SCALE = D ** -0.5
HPC = 6                 # heads per core
PAIRS = 3
P = 128
CO = C // P             # 6 contraction chunks
NO = N // P             # 16 token chunks of 128
NI4 = 4                 # i-chunks of 512
NCORES = 8
F = HPC * D             # 384

# Schraudolph fast-exp offload to DVE: number of (window, j) tiles rerouted.
# 0 disables. Tiles are taken from late windows (see _schraud_set).
SCHRAUD = int(os.environ.get("KERNEL_SCHRAUD", "0"))
# fp16 bits of e^x ~ int16(x * 2^10/ln2 + 15*2^10 - shift)
SCHRAUD_A = float(2 ** 10 / np.log(2.0))
SCHRAUD_B = float(os.environ.get("KERNEL_SCHRAUD_B", str(15 * 2 ** 10 - 45)))


def _split_multiwaits(nc):
    """This container's walrus accepts at most ONE sync-wait per instruction.

    Split any instruction carrying N>1 waits into (N-1) single-wait NOPs on
    the same engine queue placed immediately before it (engine queues are
    FIFO, so the semantics are identical)."""
    ctr = 0
    for f in nc.m.functions:
        for blk in f.blocks:
            insts = blk.instructions
            out = []
            changed = False
            for ins in insts:
                si = ins.sync_info
                if si is not None and len(si.on_wait) > 1:
                    changed = True
                    waits = list(si.on_wait)
                    for ww in waits[:-1]:
                        nop = mybir.InstNoOp(name=f"zzsplitw_{ctr}", ins=[], outs=[])
                        ctr += 1
                        nop.engine = ins.engine
                        nop.sync_info = mybir.SyncInfo(on_wait=[ww], on_update=[])
                        out.append(nop)
                    ins.sync_info = mybir.SyncInfo(
                        on_wait=[waits[-1]], on_update=list(si.on_update)
                    )
                out.append(ins)
            if changed:
                blk.instructions = out
    return nc


def _schraud_set():
    """(widx, j) tiles whose exp runs as DVE Schraudolph instead of Act exp.

    Spread across late windows, avoiding j==0/15 (PSUM start/stop edges are
    innocuous but keep the pattern simple) - per output row only a slice of
    attention positions is approximated, keeping the error ~ the measured
    one-pair level (7.5e-3)."""
    s = set()
    if SCHRAUD <= 0:
        return s
    picks = []
    for widx in range(11, -1, -1):
        for j in (5, 11, 8, 2, 14):
            picks.append((widx, j))
    for t in picks[:SCHRAUD]:
        s.add(t)
    return s


def _emit(nc, tc, ctx):
    xcd = nc.dram_tensor("xcd", [CO, P, N], F16, kind="ExternalInput").ap()
    wqd = nc.dram_tensor("wqd", [CO, P, 3 * F], F16, kind="ExternalInput").ap()
    wpd = nc.dram_tensor("wpd", [PAIRS, P, C], F16, kind="ExternalInput").ap()
    identd = nc.dram_tensor("identd", [P, P], F16, kind="ExternalInput").ap()
    outd = nc.dram_tensor("outd", [N, C], F32, kind="ExternalOutput").ap()

    persist = ctx.enter_context(tc.tile_pool(name="persist", bufs=1))
    xs = [persist.tile([P, N], F16, tag=f"xs{co}", name=f"xs{co}") for co in range(CO)]
    ws = [persist.tile([P, 3 * F], F16, tag=f"ws{co}", name=f"ws{co}")
          for co in range(CO)]
    qs = [persist.tile([P, N], F16, tag=f"qs{pr}", name=f"qs{pr}")
          for pr in range(PAIRS)]
    ks_ = [persist.tile([P, N], F16, tag=f"ks{pr}", name=f"ks{pr}")
           for pr in range(PAIRS)]
    vs = [persist.tile([P, HPC, D + 1], F16, tag=f"vs{no}", name=f"vs{no}")
          for no in range(NO)]
    ots = [persist.tile([P, N], F16, tag=f"ots{pr}", name=f"ots{pr}")
           for pr in range(PAIRS)]
    wps = persist.tile([P, PAIRS, C], F16, tag="wps")
    ident = persist.tile([P, P], F16, tag="ident")

    # exp table warm: pulls the ACT table load into the DMA lead-in window
    warm = persist.tile([P, 8], F32, tag="warm")
    nc.vector.memset(warm, 1.0)
    expwarm = persist.tile([P, 8], F32, tag="expwarm")
    nc.scalar.activation(
        out=expwarm, in_=warm, func=mybir.ActivationFunctionType.Exp, scale=1.0
    )
    for no in range(NO):
        nc.vector.memset(vs[no][:, :, D:D + 1], 1.0)

    # Input DMAs. Order = SP-queue order: minimal first-window deps first.
    nc.sync.dma_start(out=ident, in_=identd)
    for co in range(CO):
        nc.sync.dma_start(out=ws[co][:, 0:256], in_=wqd[co, :, 0:256])
        nc.sync.dma_start(out=xs[co][:, 0:512], in_=xcd[co, :, 0:512])
    for co in range(CO):
        nc.sync.dma_start(out=ws[co][:, 256:1152], in_=wqd[co, :, 256:1152])
        nc.sync.dma_start(out=xs[co][:, 512:1024], in_=xcd[co, :, 512:1024])
    for n4 in (2, 3):
        for co in range(CO):
            nc.sync.dma_start(
                out=xs[co][:, n4 * 512:(n4 + 1) * 512],
                in_=xcd[co, :, n4 * 512:(n4 + 1) * 512],
            )
    for pr in range(PAIRS):
        nc.sync.dma_start(out=wps[:, pr, :], in_=wpd[pr, :, :])

    stp = ctx.enter_context(tc.tile_pool(name="stp", bufs=2, space="PSUM"))
    pop_ = ctx.enter_context(tc.tile_pool(name="pop", bufs=2, space="PSUM"))
    mmp = ctx.enter_context(tc.tile_pool(name="mmp", bufs=2, space="PSUM"))
    ptp = ctx.enter_context(tc.tile_pool(name="ptp", bufs=18))
    otnp = ctx.enter_context(tc.tile_pool(name="otnp", bufs=3))
    rzp = ctx.enter_context(tc.tile_pool(name="rzp", bufs=4))
    osbp = ctx.enter_context(tc.tile_pool(name="osbp", bufs=2))

    def emit_qk(pr, qk, n4):
        pq = mmp.tile([P, 512], F32, tag="mm", name=f"pq_{pr}{qk}{n4}")
        col = pr * 256 + qk * 128
        for co in range(CO):
            nc.tensor.matmul(
                pq,
                ws[co][:, col:col + 128],
                xs[co][:, n4 * 512:(n4 + 1) * 512],
                start=(co == 0),
                stop=(co == CO - 1),
            )
        dst = (qs if qk == 0 else ks_)[pr]
        nc.vector.tensor_copy(out=dst[:, n4 * 512:(n4 + 1) * 512], in_=pq)

    def emit_v(no):
        pv = mmp.tile([P, 384], F32, tag="mm", name=f"pv_{no}")
        for co in range(CO):
            nc.tensor.matmul(
                pv,
                xs[co][:, no * 128:(no + 1) * 128],
                ws[co][:, 768:1152],
                start=(co == 0),
                stop=(co == CO - 1),
            )
        nc.vector.tensor_copy(
            out=vs[no][:, :, 0:D], in_=pv.rearrange("p (h d) -> p h d", h=HPC)
        )

    def emit_proj(no):
        osb = osbp.tile([P, C], F32, tag="osb", name=f"osb_{no}")
        for half in range(2):
            pp = mmp.tile([P, 384], F32, tag="mm", name=f"pp_{no}_{half}")
            for p3 in range(PAIRS):
                nc.tensor.matmul(
                    pp,
                    ots[p3][:, no * 128:(no + 1) * 128],
                    wps[:, p3, half * 384:(half + 1) * 384],
                    start=(p3 == 0),
                    stop=(p3 == PAIRS - 1),
                )
            nc.vector.tensor_copy(out=osb[:, half * 384:(half + 1) * 384], in_=pp)
        nc.sync.dma_start(out=outd[no * 128:(no + 1) * 128, :], in_=osb)

    # -- filler queue: (cost_ns, deadline slot or None, fn) --
    # Slots are linearized (widx*16 + j); pop_fillers(s) runs at the END of
    # slot s, so a filler a consumer at slot s depends on must carry deadline
    # <= s-1 (emission order defines both engine-queue order and the tile
    # dependency graph - a filler emitted after its consumer is a race).
    fillers = []
    fidx = [0]
    credit = [0.0]

    def add_filler(cost, dl, fn):
        fillers.append((cost, dl, fn))

    def pop_fillers(now):
        while fidx[0] < len(fillers):
            cost, dl, fn = fillers[fidx[0]]
            due = dl is not None and dl <= now
            if not due:
                later_due = any(
                    d is not None and d <= now for _, d, _ in fillers[fidx[0]:]
                )
                if not later_due and credit[0] < cost:
                    break
            fn()
            credit[0] -= cost
            fidx[0] += 1

    def flush_fillers():
        while fidx[0] < len(fillers):
            _, _, fn = fillers[fidx[0]]
            fn()
            fidx[0] += 1

    schraud = _schraud_set()
    ptiles = {}

    def emit_drain(w, po_a, po_b):
        ppr, pi4 = w
        for half, po in ((0, po_a), (1, po_b)):
            po_r = po.rearrange("p (r z) -> p r z", z=65)
            rz = rzp.tile([P, 4], F32, tag="rz", name=f"rz_{ppr}{pi4}{half}")
            nc.vector.reciprocal(out=rz, in_=po_r[:, :, 64])
            for ibh in range(2):
                ib = half * 2 + ibh
                otn = otnp.tile([P, P], F16, tag="otn", name=f"otn_{ppr}{pi4}{ib}")
                nc.vector.tensor_tensor(
                    out=otn,
                    in0=po_r[:, 2 * ibh:2 * ibh + 2, 0:64],
                    in1=rz[:, 2 * ibh:2 * ibh + 2].rearrange(
                        "p (r one) -> p r one", one=1
                    ).to_broadcast([P, 2, 64]),
                    op=mybir.AluOpType.mult,
                )
                tp = mmp.tile([P, P], F16, tag="mm", name=f"tp_{ppr}{pi4}{ib}")
                nc.tensor.transpose(tp, otn, ident)
                blk = pi4 * 4 + ib
                nc.vector.tensor_copy(
                    out=ots[ppr][:, blk * 128:(blk + 1) * 128], in_=tp
                )
        if ppr == PAIRS - 1:
            for no in range(4 * pi4, 4 * pi4 + 4):
                add_filler(2000, None, lambda no=no: emit_proj(no))

    def window(widx, cur, prev, prev_pos):
        if prev is not None:
            po_a = pop_.tile([P, 260], F32, tag="po", name=f"poa_{prev[0]}{prev[1]}")
            po_b = pop_.tile([P, 260], F32, tag="po", name=f"pob_{prev[0]}{prev[1]}")
        if cur is not None:
            pr, i4 = cur
            ptiles[cur] = [None] * NO
        for j in range(NO):
            if cur is not None:
                stm = stp.tile([P, 1024], F32, tag="st", name=f"st_{pr}{i4}{j}")
                nc.tensor.matmul(
                    stm[:, 0:512],
                    ks_[pr][0:64, j * 128:(j + 1) * 128],
                    qs[pr][0:64, i4 * 512:(i4 + 1) * 512],
                    start=True, stop=True,
                )
                nc.tensor.matmul(
                    stm[:, 512:1024],
                    ks_[pr][64:128, j * 128:(j + 1) * 128],
                    qs[pr][64:128, i4 * 512:(i4 + 1) * 512],
                    start=True, stop=True,
                )
                pt = ptp.tile([P, 1024], F16, tag="pt", name=f"ptile_{pr}{i4}{j}")
                ptiles[cur][j] = pt
                if (widx, j) in schraud:
                    # fast exp on DVE: fp16 bits of e^(SCALE*s) via affine +
                    # int16 convert; bitcast back to f16 is free
                    nc.vector.tensor_scalar(
                        out=pt.bitcast(I16),
                        in0=stm,
                        scalar1=float(SCALE * SCHRAUD_A),
                        scalar2=SCHRAUD_B,
                        op0=mybir.AluOpType.mult,
                        op1=mybir.AluOpType.add,
                    )
                else:
                    nc.scalar.activation(
                        out=pt, in_=stm,
                        func=mybir.ActivationFunctionType.Exp, scale=SCALE,
                    )
            if prev is not None:
                ppr = prev[0]
                ptj = ptiles[prev][j]
                # one accumulation group per po bank: start zeroes the whole
                # 2KB zero region, so only the first write starts and only
                # the last stops
                for ib in range(4):
                    po = po_a if ib < 2 else po_b
                    for h in range(2):
                        r = (ib % 2) * 2 + h
                        nc.tensor.matmul(
                            po[:, r * 65:(r + 1) * 65],
                            ptj[:, h * 512 + ib * 128: h * 512 + (ib + 1) * 128],
                            vs[j][:, 2 * ppr + h, 0:65],
                            start=(j == 0 and r == 0),
                            stop=(j == NO - 1 and r == 3),
                        )
            credit[0] = min(credit[0] + (280 if cur and prev else
                                         570 if cur else 710), 2600)
            pop_fillers(widx * NO + j)
        if prev is not None:
            emit_drain(prev, po_a, po_b)
            del ptiles[prev]

    # -- emission schedule --
    # front: minimal deps for window (0,0)
    emit_qk(0, 0, 0)
    emit_qk(0, 1, 0)
    QK = 1300
    VC = 1450
    for n4 in (1, 2, 3):
        # k chunk n4 feeds S^T(0,0,j=4*n4) at slot 4*n4
        add_filler(QK, 4 * n4 - 2, lambda n4=n4: emit_qk(0, 1, n4))
    for no in (0, 1, 2):
        # v[no] feeds PV(prev=(0,0), j=no) at slot 16+no
        add_filler(VC, 13 + no, lambda no=no: emit_v(no))
    add_filler(QK, 14, lambda: emit_qk(0, 0, 1))
    for no in range(3, NO):
        add_filler(VC, 13 + no, lambda no=no: emit_v(no))
    add_filler(QK, 2 * NO - 2, lambda: emit_qk(0, 0, 2))
    add_filler(QK, 3 * NO - 2, lambda: emit_qk(0, 0, 3))
    for pr in (1, 2):
        s0 = 4 * pr * NO
        add_filler(QK, s0 - 2, lambda pr=pr: emit_qk(pr, 0, 0))
        add_filler(QK, s0 - 2, lambda pr=pr: emit_qk(pr, 1, 0))
        for n4 in (1, 2, 3):
            add_filler(QK, s0 + 4 * n4 - 2, lambda pr=pr, n4=n4: emit_qk(pr, 1, n4))
        for n4 in (1, 2, 3):
            add_filler(QK, s0 + n4 * NO - 2, lambda pr=pr, n4=n4: emit_qk(pr, 0, n4))

    wins = [(pr, i4) for pr in range(PAIRS) for i4 in range(NI4)]
    prev = None
    for widx, cur in enumerate(wins):
        window(widx, cur, prev, widx - 1)
        prev = cur
    window(len(wins), None, prev, len(wins) - 1)
    flush_fillers()


_NC_CACHE = {}


def build_bass():
    key = (SCHRAUD, SCHRAUD_B)
    if key in _NC_CACHE:
        return _NC_CACHE[key]
    nc = bass.Bass("TRN2")
    with tile.TileContext(nc) as tc:
        with ExitStack() as ctx:
            _emit(nc, tc, ctx)
    _split_multiwaits(nc)
    _NC_CACHE[key] = nc
    return nc


def make_in_maps(x, w_qkv, w_proj):
    x = np.asarray(x, dtype=np.float32)
    w_qkv = np.asarray(w_qkv, dtype=np.float32)
    w_proj = np.asarray(w_proj, dtype=np.float32)
    wq, wk, wv = w_qkv[0:C], w_qkv[C:2 * C], w_qkv[2 * C:3 * C]
    identd = np.eye(P, dtype=np.float16)
    in_maps = []
    for c in range(NCORES):
        b, g = divmod(c, 2)
        base = g * F
        cols = []
        for p3 in range(PAIRS):
            lo = base + p3 * 128
            cols.append(wq[lo:lo + 128])
            cols.append(wk[lo:lo + 128])
        cols.append(wv[base:base + F])
        wsel = np.concatenate(cols, axis=0)            # [1152, 768]
        wqd = np.ascontiguousarray(wsel.T.astype(np.float16)).reshape(CO, P, 3 * F)
        xcd = np.ascontiguousarray(x[b].T.astype(np.float16)).reshape(CO, P, N)
        wpd = np.stack(
            [
                np.ascontiguousarray(
                    w_proj[:, base + p3 * 128: base + (p3 + 1) * 128].T
                ).astype(np.float16)
                for p3 in range(PAIRS)
            ]
        )
        in_maps.append({"xcd": xcd, "wqd": wqd, "wpd": wpd, "identd": identd})
    return in_maps


def gather_output(parts, b_proj):
    """parts: 8 arrays [N, C] (head-group partials per core)."""
    outv = np.empty((B, N, C), np.float32)
    for b in range(B):
        outv[b] = parts[2 * b] + parts[2 * b + 1]
    outv += np.asarray(b_proj, dtype=np.float32)[None, None, :]
    return outv


def kernel(x, w_qkv, w_proj, b_proj, _run_kwargs=None):
    nc = build_bass()
    in_maps = make_in_maps(x, w_qkv, w_proj)
    res = bass_utils.run_bass_kernel_spmd(
        nc, in_maps, core_ids=list(range(NCORES)), **(_run_kwargs or {})
    )
    parts = [r["outd"] for r in res.results]
    outv = gather_output(parts, b_proj)
    if _run_kwargs is not None:
        kernel.last_results = res
    return outv


# revision 14
# speedup vs baseline: 1.2507x; 1.1046x over previous
"""Trainium2 Bass kernel for nn_Attention (B=4, N=2048, C=768, H=12).

Sharding: 8 cores = 4 batches x 2 head-groups (6 heads = 3 pairs each),
Megatron-style tensor parallel on heads. Each core computes qkv for its head
slice, attention, and the head-group's projection partial out [2048, 768];
the host sums the 2 group partials per batch and adds the bias.

Design (cost-model driven; matmul cost = out_free_size x 1 cycle/row for f16):
  - All matmul operands fp16 (host-cast inputs); PSUM accumulation fp32.
    Measured end-to-end rel err ~1e-3 vs fp32 reference (tolerance 2e-2).
  - x resident in SBUF as 6 [128, 2048] c-chunks, read from DRAM once.
  - S^T tiles [128 j, 1024] = two heads x 512 i, exp'd on the Act engine
    straight out of PSUM into fp16 ptiles.
  - PV in [i, d] layout: po[i, 65] += ptile[j, i-block].T @ v[j, d+ones]
    (free size 65 instead of 512 -> half the PE cost of a [d, i] PV).
    Column 64 of v is ones so row sums Z accumulate alongside.
  - PV for window (pair, i4) is deferred one window: it fills PE slack while
    the next window's exp stream keeps the Act engine (the near-critical
    engine) saturated.
  - Normalize: 1/Z per-partition (token) via DVE reciprocal + tensor_tensor
    broadcast multiply -> [i, d] fp16; PE-transpose (identity) -> ot [d, i];
    projection accumulates all 3 pairs into one PSUM tile per token block.
  - qkv / v / proj / transpose work is emitted via a deadline-driven filler
    queue into the attention windows' PE slack.
"""

import os
import sys
from contextlib import ExitStack

if "/opt/trn_rl_repo" not in sys.path:
    sys.path.insert(0, "/opt/trn_rl_repo")

import numpy as np

import concourse.bass as bass
import concourse.mybir as mybir
import concourse.tile as tile
from concourse import bass_utils

F32 = mybir.dt.float32
F16 = mybir.dt.float16
I16 = mybir.dt.int16

B, N, C = 4, 2048, 768
NH, D = 12, 64
SCALE = D ** -0.5
HPC = 6                 # heads per core
PAIRS = 3
P = 128
CO = C // P             # 6 contraction chunks
NO = N // P             # 16 token chunks of 128
NI4 = 4                 # i-chunks of 512
NCORES = 8
F = HPC * D             # 384

# Schraudolph fast-exp offload to DVE: number of (window, j) tiles rerouted.
# 0 disables. Tiles are taken from late windows (see _schraud_set).
SCHRAUD = int(os.environ.get("KERNEL_SCHRAUD", "0"))
# fp16 bits of e^x ~ int16(x * 2^10/ln2 + 15*2^10 - shift)
SCHRAUD_A = float(2 ** 10 / np.log(2.0))
SCHRAUD_B = float(os.environ.get("KERNEL_SCHRAUD_B", str(15 * 2 ** 10 - 45)))


def _split_multiwaits(nc):
    """This container's walrus accepts at most ONE sync-wait per instruction.

    Split any instruction carrying N>1 waits into (N-1) single-wait NOPs on
    the same engine queue placed immediately before it (engine queues are
    FIFO, so the semantics are identical)."""
    ctr = 0
    for f in nc.m.functions:
        for blk in f.blocks:
            insts = blk.instructions
            out = []
            changed = False
            for ins in insts:
                si = ins.sync_info
                if si is not None and len(si.on_wait) > 1:
                    changed = True
                    waits = list(si.on_wait)
                    for ww in waits[:-1]:
                        nop = mybir.InstNoOp(name=f"zzsplitw_{ctr}", ins=[], outs=[])
                        ctr += 1
                        nop.engine = ins.engine
                        nop.sync_info = mybir.SyncInfo(on_wait=[ww], on_update=[])
                        out.append(nop)
                    ins.sync_info = mybir.SyncInfo(
                        on_wait=[waits[-1]], on_update=list(si.on_update)
                    )
                out.append(ins)
            if changed:
                blk.instructions = out
    return nc


def _schraud_set():
    """(widx, j) tiles whose exp runs as DVE Schraudolph instead of Act exp.

    Spread across late windows, avoiding j==0/15 (PSUM start/stop edges are
    innocuous but keep the pattern simple) - per output row only a slice of
    attention positions is approximated, keeping the error ~ the measured
    one-pair level (7.5e-3)."""
    s = set()
    if SCHRAUD <= 0:
        return s
    picks = []
    for widx in range(11, -1, -1):
        for j in (5, 11, 8, 2, 14):
            picks.append((widx, j))
    for t in picks[:SCHRAUD]:
        s.add(t)
    return s


def _emit(nc, tc, ctx):
    xcd = nc.dram_tensor("xcd", [CO, P, N], F16, kind="ExternalInput").ap()
    wqd = nc.dram_tensor("wqd", [CO, P, 3 * F], F16, kind="ExternalInput").ap()
    wpd = nc.dram_tensor("wpd", [PAIRS, P, C], F16, kind="ExternalInput").ap()
    identd = nc.dram_tensor("identd", [P, P], F16, kind="ExternalInput").ap()
    outd = nc.dram_tensor("outd", [N, C], F32, kind="ExternalOutput").ap()

    persist = ctx.enter_context(tc.tile_pool(name="persist", bufs=1))
    xs_all = persist.tile([P, CO, N], F16, tag="xs")
    ws_all = persist.tile([P, CO, 3 * F], F16, tag="ws")
    xs = [xs_all[:, co, :] for co in range(CO)]
    ws = [ws_all[:, co, :] for co in range(CO)]
    qs = [persist.tile([P, N], F16, tag=f"qs{pr}", name=f"qs{pr}")
          for pr in range(PAIRS)]
    ks_ = [persist.tile([P, N], F16, tag=f"ks{pr}", name=f"ks{pr}")
           for pr in range(PAIRS)]
    vs = [persist.tile([P, HPC, D + 1], F16, tag=f"vs{no}", name=f"vs{no}")
          for no in range(NO)]
    ots = [persist.tile([P, N], F16, tag=f"ots{pr}", name=f"ots{pr}")
           for pr in range(PAIRS)]
    wps = persist.tile([P, PAIRS, C], F16, tag="wps")
    ident = persist.tile([P, P], F16, tag="ident")

    # exp table warm: pulls the ACT table load into the DMA lead-in window
    warm = persist.tile([P, 8], F32, tag="warm")
    nc.vector.memset(warm, 1.0)
    expwarm = persist.tile([P, 8], F32, tag="expwarm")
    nc.scalar.activation(
        out=expwarm, in_=warm, func=mybir.ActivationFunctionType.Exp, scale=1.0
    )


    # Input DMAs. Order = SP-queue order (650ns serial issue per DMA), so
    # merge aggressively: pair-0 weights in one DMA, then per-co x chunks
    # (kept separate so the first matmuls start as soon as their chunk
    # lands), then one merged DMA per remaining region.
    xcd_p = xcd.rearrange("c p n -> p c n")
    wqd_p = wqd.rearrange("c p f -> p c f")
    nc.sync.dma_start(out=ws_all[:, :, 0:256], in_=wqd_p[:, :, 0:256])
    for ch in range(3):
        nc.sync.dma_start(
            out=xs_all[:, 2 * ch:2 * ch + 2, 0:512],
            in_=xcd_p[:, 2 * ch:2 * ch + 2, 0:512],
        )
    for n4 in (1, 2, 3):
        nc.sync.dma_start(
            out=xs_all[:, :, n4 * 512:(n4 + 1) * 512],
            in_=xcd_p[:, :, n4 * 512:(n4 + 1) * 512],
        )
    nc.sync.dma_start(out=ws_all[:, :, 256:1152], in_=wqd_p[:, :, 256:1152])
    nc.sync.dma_start(out=ident, in_=identd)
    nc.sync.dma_start(out=wps, in_=wpd.rearrange("r p c -> p r c"))

    stp = ctx.enter_context(tc.tile_pool(name="stp", bufs=2, space="PSUM"))
    pop_ = ctx.enter_context(tc.tile_pool(name="pop", bufs=2, space="PSUM"))
    mmp = ctx.enter_context(tc.tile_pool(name="mmp", bufs=2, space="PSUM"))
    ptp = ctx.enter_context(tc.tile_pool(name="ptp", bufs=18))
    otnp = ctx.enter_context(tc.tile_pool(name="otnp", bufs=3))
    rzp = ctx.enter_context(tc.tile_pool(name="rzp", bufs=4))
    osbp = ctx.enter_context(tc.tile_pool(name="osbp", bufs=2))

    # PE pre-warm: tiny dummy matmuls keep the PE continuously busy through
    # the DMA lead-in so the p-state ramp (2x slower cycles for the first
    # 3us of busy time) is spent before the first real matmul. They rotate
    # through the mm slots ahead of any real user, costing no extra banks.
    for i in range(34):
        pw = mmp.tile([8, 8], F32, tag="mm", name=f"pw{i}")
        nc.tensor.matmul(pw, warm[:, 0:8], warm[:, 0:8], start=True, stop=True)

    # vs ones-columns: needed only by the first PV (slot 16); emitted here so
    # the DVE queue serves the front q/k evacuations first
    for no in range(NO):
        nc.vector.memset(vs[no][:, :, D:D + 1], 1.0)

    def qk_step(pr, qk, n4, co, box):
        if co == 0:
            box["pq"] = mmp.tile([P, 512], F32, tag="mm", name=f"pq_{pr}{qk}{n4}")
        col = pr * 256 + qk * 128
        nc.tensor.matmul(
            box["pq"],
            ws[co][:, col:col + 128],
            xs[co][:, n4 * 512:(n4 + 1) * 512],
            start=(co == 0),
            stop=(co == CO - 1),
        )
        if co == CO - 1:
            dst = (qs if qk == 0 else ks_)[pr]
            nc.vector.tensor_copy(out=dst[:, n4 * 512:(n4 + 1) * 512], in_=box["pq"])

    def emit_qk(pr, qk, n4):
        box = {}
        for co in range(CO):
            qk_step(pr, qk, n4, co, box)

    def qk_step_rest(pr, co, box):
        # k chunk n4=0 columns 128:512 (first 128 handled by the front split)
        if co == 0:
            box["pq"] = mmp.tile([P, 384], F32, tag="mm", name=f"pkr_{pr}")
        nc.tensor.matmul(
            box["pq"],
            ws[co][:, pr * 256 + 128:pr * 256 + 256],
            xs[co][:, 128:512],
            start=(co == 0),
            stop=(co == CO - 1),
        )
        if co == CO - 1:
            nc.vector.tensor_copy(out=ks_[pr][:, 128:512], in_=box["pq"])

    def v_step(p3, no, co, box):
        if co == 0:
            box["pv"] = mmp.tile([P, 128], F32, tag="mm", name=f"pv_{p3}_{no}")
        nc.tensor.matmul(
            box["pv"],
            xs[co][:, no * 128:(no + 1) * 128],
            ws[co][:, 768 + p3 * 128:768 + (p3 + 1) * 128],
            start=(co == 0),
            stop=(co == CO - 1),
        )
        if co == CO - 1:
            nc.vector.tensor_copy(
                out=vs[no][:, 2 * p3:2 * p3 + 2, 0:D],
                in_=box["pv"].rearrange("p (h d) -> p h d", h=2),
            )

    def add_qk_fillers(pr, qk, n4, dl):
        box = {}
        for co in range(CO):
            add_filler(215, dl, lambda pr=pr, qk=qk, n4=n4, co=co, box=box:
                       qk_step(pr, qk, n4, co, box))

    def add_v_fillers(p3, no, dl):
        box = {}
        for co in range(CO):
            add_filler(120, dl, lambda p3=p3, no=no, co=co, box=box:
                       v_step(p3, no, co, box))

    def emit_proj(no):
        osb = osbp.tile([P, C], F32, tag="osb", name=f"osb_{no}")
        for half in range(2):
            pp = mmp.tile([P, 384], F32, tag="mm", name=f"pp_{no}_{half}")
            for p3 in range(PAIRS):
                nc.tensor.matmul(
                    pp,
                    ots[p3][:, no * 128:(no + 1) * 128],
                    wps[:, p3, half * 384:(half + 1) * 384],
                    start=(p3 == 0),
                    stop=(p3 == PAIRS - 1),
                )
            nc.vector.tensor_copy(out=osb[:, half * 384:(half + 1) * 384], in_=pp)
        nc.sync.dma_start(out=outd[no * 128:(no + 1) * 128, :], in_=osb)

    # -- filler queue: (cost_ns, deadline slot or None, fn) --
    # Slots are linearized (widx*16 + j); pop_fillers(s) runs at the END of
    # slot s, so a filler a consumer at slot s depends on must carry deadline
    # <= s-1 (emission order defines both engine-queue order and the tile
    # dependency graph - a filler emitted after its consumer is a race).
    fillers = []
    fidx = [0]
    credit = [0.0]

    def add_filler(cost, dl, fn):
        fillers.append((cost, dl, fn))

    def pop_fillers(now):
        while fidx[0] < len(fillers):
            cost, dl, fn = fillers[fidx[0]]
            due = dl is not None and dl <= now
            if not due:
                later_due = any(
                    d is not None and d <= now for _, d, _ in fillers[fidx[0]:]
                )
                if not later_due and credit[0] < cost:
                    break
            fn()
            credit[0] = max(credit[0] - cost, -1200.0)
            fidx[0] += 1

    def flush_fillers():
        while fidx[0] < len(fillers):
            _, _, fn = fillers[fidx[0]]
            fn()
            fidx[0] += 1

    schraud = _schraud_set()
    ptiles = {}

    def emit_drain(w, po_a, po_b):
        ppr, pi4 = w
        for half, po in ((0, po_a), (1, po_b)):
            po_r = po.rearrange("p (r z) -> p r z", z=65)
            rz = rzp.tile([P, 4], F32, tag="rz", name=f"rz_{ppr}{pi4}{half}")
            nc.vector.reciprocal(out=rz, in_=po_r[:, :, 64])
            for ibh in range(2):
                ib = half * 2 + ibh
                otn = otnp.tile([P, P], F16, tag="otn", name=f"otn_{ppr}{pi4}{ib}")
                nc.vector.tensor_tensor(
                    out=otn,
                    in0=po_r[:, 2 * ibh:2 * ibh + 2, 0:64],
                    in1=rz[:, 2 * ibh:2 * ibh + 2].rearrange(
                        "p (r one) -> p r one", one=1
                    ).to_broadcast([P, 2, 64]),
                    op=mybir.AluOpType.mult,
                )
                tp = mmp.tile([P, P], F16, tag="mm", name=f"tp_{ppr}{pi4}{ib}")
                nc.tensor.transpose(tp, otn, ident)
                blk = pi4 * 4 + ib
                nc.vector.tensor_copy(
                    out=ots[ppr][:, blk * 128:(blk + 1) * 128], in_=tp
                )
        if ppr == PAIRS - 1:
            for no in range(4 * pi4, 4 * pi4 + 4):
                add_filler(1000, None, lambda no=no: emit_proj(no))

    def window(widx, cur, prev, prev_pos):
        if prev is not None:
            po_a = pop_.tile([P, 260], F32, tag="po", name=f"poa_{prev[0]}{prev[1]}")
            po_b = pop_.tile([P, 260], F32, tag="po", name=f"pob_{prev[0]}{prev[1]}")
        if cur is not None:
            pr, i4 = cur
            ptiles[cur] = [None] * NO
        for j in range(NO):
            if cur is not None:
                stm = stp.tile([P, 1024], F32, tag="st", name=f"st_{pr}{i4}{j}")
                nc.tensor.matmul(
                    stm[:, 0:512],
                    ks_[pr][0:64, j * 128:(j + 1) * 128],
                    qs[pr][0:64, i4 * 512:(i4 + 1) * 512],
                    start=True, stop=True,
                )
                nc.tensor.matmul(
                    stm[:, 512:1024],
                    ks_[pr][64:128, j * 128:(j + 1) * 128],
                    qs[pr][64:128, i4 * 512:(i4 + 1) * 512],
                    start=True, stop=True,
                )
                pt = ptp.tile([P, 1024], F16, tag="pt", name=f"ptile_{pr}{i4}{j}")
                ptiles[cur][j] = pt
                if (widx, j) in schraud:
                    # fast exp on DVE: fp16 bits of e^(SCALE*s) via affine +
                    # int16 convert; bitcast back to f16 is free
                    nc.vector.tensor_scalar(
                        out=pt.bitcast(I16),
                        in0=stm,
                        scalar1=float(SCALE * SCHRAUD_A),
                        scalar2=SCHRAUD_B,
                        op0=mybir.AluOpType.mult,
                        op1=mybir.AluOpType.add,
                    )
                else:
                    nc.scalar.activation(
                        out=pt, in_=stm,
                        func=mybir.ActivationFunctionType.Exp, scale=SCALE,
                    )
            if prev is not None:
                ppr = prev[0]
                ptj = ptiles[prev][j]
                # one accumulation group per po bank: start zeroes the whole
                # 2KB zero region, so only the first write starts and only
                # the last stops
                for ib in range(4):
                    po = po_a if ib < 2 else po_b
                    for h in range(2):
                        r = (ib % 2) * 2 + h
                        nc.tensor.matmul(
                            po[:, r * 65:(r + 1) * 65],
                            ptj[:, h * 512 + ib * 128: h * 512 + (ib + 1) * 128],
                            vs[j][:, 2 * ppr + h, 0:65],
                            start=(j == 0 and r == 0),
                            stop=(j == NO - 1 and r == 3),
                        )
            credit[0] = min(credit[0] + (280 if cur and prev else
                                         570 if cur else 710), 2600)
            pop_fillers(widx * NO + j)
        if prev is not None:
            emit_drain(prev, po_a, po_b)
            del ptiles[prev]

    # -- emission schedule --
    # front: minimal deps for window (0,0), q/k interleaved per co so the
    # last matmul waits only the last x-chunk DMA. k is split so the first
    # 128-token chunk (all S^T(0,0,0) needs) lands before the k remainder.
    pk0 = mmp.tile([P, 128], F32, tag="mm", name="pk0")
    boxq, boxk = {}, {}
    for co in range(CO):
        qk_step(0, 0, 0, co, boxq)
        nc.tensor.matmul(
            pk0, ws[co][:, 128:256], xs[co][:, 0:128],
            start=(co == 0), stop=(co == CO - 1),
        )
    nc.vector.tensor_copy(out=ks_[0][:, 0:128], in_=pk0)
    for co in range(CO):
        qk_step_rest(0, co, boxk)
    for n4 in (1, 2, 3):
        # k chunk n4 feeds S^T(0,0,j=4*n4) at slot 4*n4
        add_qk_fillers(0, 1, n4, max(0, 4 * n4 - 4))
    for no in (0, 1, 2):
        # v[p3=0, no] feeds PV(prev=(0,0), j=no) at slot 16+no
        add_v_fillers(0, no, 13 + no)
    add_qk_fillers(0, 0, 1, 14)
    for no in range(3, NO):
        add_v_fillers(0, no, 13 + no)
    add_qk_fillers(0, 0, 2, 2 * NO - 2)
    add_qk_fillers(0, 0, 3, 3 * NO - 2)
    for pr in (1, 2):
        s0 = 4 * pr * NO
        add_qk_fillers(pr, 0, 0, s0 - 2)
        add_qk_fillers(pr, 1, 0, s0 - 2)
        for n4 in (1, 2, 3):
            add_qk_fillers(pr, 1, n4, s0 + 4 * n4 - 2)
        for n4 in (1, 2, 3):
            add_qk_fillers(pr, 0, n4, s0 + n4 * NO - 2)
        for no in range(NO):
            # v[pr, no] feeds PV(prev=(pr,0), j=no) at slot s0+16+no; due
            # inside the pair's own first window (quiet), keeping both the
            # congested pair-0 phase and the proj-carrying last windows free
            add_v_fillers(pr, no, s0 - 2 + no)

    wins = [(pr, i4) for pr in range(PAIRS) for i4 in range(NI4)]
    prev = None
    for widx, cur in enumerate(wins):
        window(widx, cur, prev, widx - 1)
        prev = cur
    window(len(wins), None, prev, len(wins) - 1)
    flush_fillers()


_NC_CACHE = {}


def build_bass():
    key = (SCHRAUD, SCHRAUD_B)
    if key in _NC_CACHE:
        return _NC_CACHE[key]
    nc = bass.Bass("TRN2")
    with tile.TileContext(nc) as tc:
        with ExitStack() as ctx:
            _emit(nc, tc, ctx)
    _split_multiwaits(nc)
    _NC_CACHE[key] = nc
    return nc


def make_in_maps(x, w_qkv, w_proj):
    x = np.asarray(x, dtype=np.float32)
    w_qkv = np.asarray(w_qkv, dtype=np.float32)
    w_proj = np.asarray(w_proj, dtype=np.float32)
    wq, wk, wv = w_qkv[0:C], w_qkv[C:2 * C], w_qkv[2 * C:3 * C]
    identd = np.eye(P, dtype=np.float16)
    in_maps = []
    for c in range(NCORES):
        b, g = divmod(c, 2)
        base = g * F
        cols = []
        for p3 in range(PAIRS):
            lo = base + p3 * 128
            cols.append(wq[lo:lo + 128])
            cols.append(wk[lo:lo + 128])
        cols.append(wv[base:base + F])
        wsel = np.concatenate(cols, axis=0)            # [1152, 768]
        wqd = np.ascontiguousarray(wsel.T.astype(np.float16)).reshape(CO, P, 3 * F)
        xcd = np.ascontiguousarray(x[b].T.astype(np.float16)).reshape(CO, P, N)
        wpd = np.stack(
            [
                np.ascontiguousarray(
                    w_proj[:, base + p3 * 128: base + (p3 + 1) * 128].T
                ).astype(np.float16)
                for p3 in range(PAIRS)
            ]
        )
        in_maps.append({"xcd": xcd, "wqd": wqd, "wpd": wpd, "identd": identd})
    return in_maps


def gather_output(parts, b_proj):
    """parts: 8 arrays [N, C] (head-group partials per core)."""
    outv = np.empty((B, N, C), np.float32)
    for b in range(B):
        outv[b] = parts[2 * b] + parts[2 * b + 1]
    outv += np.asarray(b_proj, dtype=np.float32)[None, None, :]
    return outv


def kernel(x, w_qkv, w_proj, b_proj, _run_kwargs=None):
    nc = build_bass()
    in_maps = make_in_maps(x, w_qkv, w_proj)
    res = bass_utils.run_bass_kernel_spmd(
        nc, in_maps, core_ids=list(range(NCORES)), **(_run_kwargs or {})
    )
    parts = [r["outd"] for r in res.results]
    outv = gather_output(parts, b_proj)
    if _run_kwargs is not None:
        kernel.last_results = res
    return outv
